# revision 17
# baseline (speedup 1.0000x reference)
"""Trainium2 Bass kernel for rank-1-logit self attention.

Reference computation (per batch b):
    q = X @ rot.sum(axis=1)            # [S]   (einsum broadcast collapses the k=3 dim)
    k = X @ ent                        # [S]
    logits[s,t] = q[s] * k[t] / sqrt(d)     (rank-1!)
    out = softmax(logits, axis=-1) @ X

Strategy: data-parallel over batch, one batch per NeuronCore (B=8, 8 cores).
Per core, with rows sorted by q (host-side permutation):
    E_T[t, s] = exp(q[s] * (k[t] - K_half) / sqrt(d))    K = kmin for the low-q
                half, kmax for the high-q half -> exponent <= ~4, never overflows
                and equals exact row-max subtraction up to a per-row constant
                that cancels in the normalization.
    O^T[d, s] = sum_t X[t, d] * E_T[t, s]   (PE matmul, X chunks stationary)
    Z[s]      = sum_t E_T[t, s]             (PE matmul, ones stationary)
    out[s, d] = O^T[d, s] / Z[s]            (PE transpose + per-partition scale)

The exp is a single ScalarE activation per (t-chunk, s-half): the logit
multiply rides the activation's per-partition `scale` operand for free.
"""

import os
import sys

import numpy as np

for _p in ("/opt/trn_rl_repo",):
    if os.path.isdir(_p) and _p not in sys.path:
        sys.path.append(_p)

import concourse.bass as bass
import concourse.mybir as mybir
import concourse.tile as tile
from concourse import bacc
from concourse.bass_utils import run_bass_kernel_spmd
from concourse.masks import make_identity

B, S, D = 8, 2048, 128
TC = S // 128  # t-chunks of 128
NJ = S // 512  # 512-wide s-chunks (PSUM bank width in fp32)
SQRT_D = float(np.sqrt(np.float32(D)))

F32 = mybir.dt.float32
F32R = mybir.dt.float32r  # full-rate fp32 matmul mode (N>=256)


def _build(mm_dtype=F32R, repeats=1):
    nc = bacc.Bacc("TRN2", target_bir_lowering=False, debug=False)
    x = nc.dram_tensor("x", [S, D], mm_dtype, kind="ExternalInput")
    qb = nc.dram_tensor("qb", [128, S], F32, kind="ExternalInput")
    scl = nc.dram_tensor("scl", [128, 2 * TC], F32, kind="ExternalInput")
    o = nc.dram_tensor("o", [S, D], F32, kind="ExternalOutput")
    zscratch = nc.dram_tensor("zs", [1, S], F32)

    EXP = mybir.ActivationFunctionType.Exp

    with tile.TileContext(nc) as tc:
        for rep in range(repeats):
            _emit_body(nc, tc, x, qb, scl, o, zscratch, mm_dtype, EXP, rep)
    nc.compile()
    return nc


def _emit_body(nc, tc, x, qb, scl, o, zscratch, mm_dtype, EXP, rep):
    H = S // 2  # 1024: one sorted-q half = one K-group = one pipeline stage
    with (
            tc.tile_pool(name=f"const{rep}", bufs=1) as cpool,
            tc.tile_pool(name=f"xw{rep}", bufs=TC) as xpool,
            tc.tile_pool(name=f"e{rep}", bufs=3) as epool,
            tc.tile_pool(name=f"drain{rep}", bufs=2) as dpool,
            tc.tile_pool(name=f"outp{rep}", bufs=4) as opool,
            tc.tile_pool(name=f"tr{rep}", bufs=4, space="PSUM") as trpool,
    ):
        qbap = qb.ap()
        xap = x.ap()
        oap = o.ap()
        zsap = zscratch.ap()

        scl_sb = cpool.tile([128, 2 * TC], F32)
        nc.sync.dma_start(scl_sb[:], scl.ap())
        ident = cpool.tile([128, 128], F32)
        make_identity(nc, ident[:])
        ones_f = cpool.tile([128, 1], F32)
        nc.vector.memset(ones_f[:], 1.0)
        ones = cpool.tile([128, 1], mm_dtype)
        nc.scalar.copy(ones[:], ones_f[:])

        x_sb = []
        for c in range(TC):
            xt = xpool.tile([128, D], mm_dtype, name=f"x{rep}_{c}")
            nc.sync.dma_start(xt[:], xap[c * 128 : (c + 1) * 128, :])
            x_sb.append(xt)

        for h in range(2):  # half 0: K=kmin (low q), half 1: K=kmax (high q)
            q_sb = cpool.tile([128, H], F32, name=f"q{rep}_{h}")
            nc.sync.dma_start(q_sb[:], qbap[:, h * H : (h + 1) * H])

            with (
                tc.tile_pool(name=f"om{rep}_{h}", bufs=1, space="PSUM") as ompool,
                tc.tile_pool(name=f"zp{rep}_{h}", bufs=1, space="PSUM") as zpool,
            ):
                om_ps = ompool.tile([128, H], F32, name=f"om{rep}_{h}")  # 2 banks
                z_ps = zpool.tile([1, H], F32, name=f"z{rep}_{h}")  # 2 banks

                for c in range(TC):
                    e_sb = epool.tile([128, H], mm_dtype, name=f"e{rep}_{h}_{c}", tag="e")
                    nc.scalar.activation(
                        e_sb[:], q_sb[:], EXP,
                        bias=0.0, scale=scl_sb[:, h * TC + c : h * TC + c + 1],
                    )
                    for j in range(2):
                        rhs = e_sb[:, j * 512 : (j + 1) * 512]
                        nc.tensor.matmul(
                            om_ps[:, j * 512 : (j + 1) * 512], x_sb[c][:], rhs,
                            start=(c == 0), stop=(c == TC - 1),
                        )
                        nc.tensor.matmul(
                            z_ps[0:1, j * 512 : (j + 1) * 512], ones[:], rhs,
                            start=(c == 0), stop=(c == TC - 1),
                        )

                # drain this half (overlaps the other half's c-loop)
                z_sb = dpool.tile([1, H], F32, name=f"zs{rep}_{h}", tag="zs")
                nc.scalar.copy(z_sb[:], z_ps[:])
                nc.sync.dma_start(zsap[:, h * H : (h + 1) * H], z_sb[:])
                ot = []
                for j in range(2):
                    t = dpool.tile([128, 512], F32, name=f"ot{rep}_{h}_{j}", tag=f"ot{j}")
                    src = om_ps[:, j * 512 : (j + 1) * 512]
                    if j == 0:
                        nc.vector.tensor_copy(t[:], src)
                    else:
                        nc.scalar.copy(t[:], src)
                    ot.append(t)

            # Z [1, 1024] -> [128, 8] via DRAM bounce (partition-aligned with
            # the transposed output tiles)
            z2 = dpool.tile([128, TC // 2], F32, name=f"z2{rep}_{h}", tag="z2")
            nc.sync.dma_start(
                z2[:],
                zsap[:, h * H : (h + 1) * H].rearrange("a (i p) -> p (a i)", p=128),
            )
            z2r = dpool.tile([128, TC // 2], F32, name=f"z2r{rep}_{h}", tag="z2r")
            nc.vector.reciprocal(z2r[:], z2[:])

            for i in range(TC // 2):
                s0 = h * H + i * 128
                tr_ps = trpool.tile([128, 128], F32, name=f"tr{rep}_{h}_{i}", tag="tr")
                nc.tensor.transpose(
                    tr_ps[:], ot[i // 4][:, (i % 4) * 128 : (i % 4 + 1) * 128], ident[:]
                )
                o_sb = opool.tile([128, 128], F32, name=f"o{rep}_{h}_{i}", tag="o")
                if i % 2 == 0:
                    nc.vector.tensor_scalar_mul(o_sb[:], tr_ps[:], z2r[:, i : i + 1])
                else:
                    nc.scalar.mul(o_sb[:], tr_ps[:], z2r[:, i : i + 1])
                nc.sync.dma_start(oap[s0 : s0 + 128, :], o_sb[:])


_NC_CACHE = {}


def _get_nc(mm_dtype=F32R):
    key = str(mm_dtype)
    if key not in _NC_CACHE:
        _NC_CACHE[key] = _build(mm_dtype)
    return _NC_CACHE[key]


def build_in_maps(inputs, rotation_params, entangle_params, mm_dtype=F32R):
    X = np.ascontiguousarray(np.asarray(inputs, dtype=np.float32))
    rot = np.asarray(rotation_params, dtype=np.float32).reshape(D, 3)
    ent = np.asarray(entangle_params, dtype=np.float32).reshape(D)
    rsum = rot.sum(axis=1)
    x_np_dtype = mybir.dt.np(mm_dtype)

    in_maps = []
    perms = []
    for b in range(B):
        q = X[b] @ rsum
        k = X[b] @ ent
        perm = np.argsort(q, kind="stable")
        qp = q[perm]
        scl = np.empty((128, 2 * TC), dtype=np.float32)
        scl[:, 0:TC] = ((k - k.min()) / SQRT_D).reshape(TC, 128).T  # low-q half
        scl[:, TC : 2 * TC] = ((k - k.max()) / SQRT_D).reshape(TC, 128).T  # high-q half
        in_maps.append(
            {
                "x": np.ascontiguousarray(X[b].astype(x_np_dtype)),
                "qb": np.ascontiguousarray(np.broadcast_to(qp, (128, S))),
                "scl": scl,
            }
        )
        perms.append(perm)
    return in_maps, perms, X


def kernel(inputs, rotation_params, entangle_params, _trace=False, _mm_dtype=None):
    mm_dtype = _mm_dtype if _mm_dtype is not None else F32R
    in_maps, perms, X = build_in_maps(inputs, rotation_params, entangle_params, mm_dtype)
    nc = _get_nc(mm_dtype)
    res = run_bass_kernel_spmd(nc, in_maps, core_ids=list(range(B)), trace=_trace)

    out = np.empty_like(X)
    for b in range(B):
        out[b][perms[b]] = res.results[b]["o"]
    if _trace:
        kernel.last_exec_time_ns = res.exec_time_ns
        kernel.last_results = res
    return out
